# revision 71
# baseline (speedup 1.0000x reference)
"""Multi-head attention (B=16, T=1024, D=768, H=12) on 8 TRN2 NeuronCores.

Strategy: pure data parallelism over the batch dim (2 batches per core, no
collectives). Per core, a Tile kernel computes the full attention block:

  qkv = x @ Wqkv.T + b            (q,k produced transposed [o, T]; v normal [T, o])
  scoresT = (k_h qT_h) * scale    ([j, i] layout; the 2 heads of a pair run as
                                   row-tiled CONCURRENT matmuls into the two
                                   halves of one 2-bank PSUM tile)
  expT = exp(scoresT)             (one [128,1024] ACTIVATE per j-tile covers both
                                   heads: (N+352)/1.2 amortizes the 352-cyc cost)
  outT_aug = v_aug.T? PV matmul   (v with appended ones col -> rows 0..63 = out,
                                   row 64 = softmax denominators)
  outT = outT_aug[:64] / sums     (reciprocal_approx_fast + bcast DMA + DVE mul)
  y = outT.T @ WprojT + b         (normal [t, e] layout, contiguous DMA out)

Perf architecture (vs the naive staged version): the attention phase is
ScalarE(exp)-bound, which starves the PE and lets the HAM clock gate
re-throttle it to 1.2 GHz. To keep the PE saturated at 2.4 GHz, independent
matmul "filler groups" (the OTHER batch element's qkv projection, and the
previous batch's output projection) are woven into the attention phase's
tensor-engine stream via a pump queue. The serial softmax-normalize chain
uses reciprocal_approx_fast (~5x faster than InstReciprocal) and runs on
DVE/GpSimd/DMA entirely off the PE critical path.
"""

import numpy as np

import concourse.bass as bass
import concourse.mybir as mybir
import concourse.tile as tile
from concourse import bacc
from concourse.bass_utils import run_bass_kernel_spmd

F32 = mybir.dt.float32
BF16 = mybir.dt.bfloat16

N_CORES = 8
B = 16
T = 1024
NH = 12
HD = 64
DIM = NH * HD
B_LOC = B // N_CORES
TC = 512  # free-dim chunk (one PSUM bank of f32)


def build_nc(b_loc=B_LOC, t=T, nh=NH):
    assert nh % 2 == 0
    dim = nh * HD
    o3 = 3 * dim
    n_dc = dim // 128      # contraction chunks over dim
    n_hp = nh // 2         # head pairs
    n_tt = t // 128        # t tiles
    scale = HD ** -0.5

    nc = bacc.Bacc()

    xT_d = nc.declare_dram_parameter("xT", [b_loc, dim, t], BF16, isOutput=False)
    wq_d = nc.declare_dram_parameter("w_qkvT", [dim, o3], BF16, isOutput=False)
    wp_d = nc.declare_dram_parameter("w_projT", [dim, dim], BF16, isOutput=False)
    bqk_d = nc.declare_dram_parameter("b_qkT", [128, 2 * n_hp], F32, isOutput=False)
    bv_d = nc.declare_dram_parameter("b_v", [128, dim], F32, isOutput=False)
    bp_d = nc.declare_dram_parameter("b_proj", [128, dim], F32, isOutput=False)
    out_d = nc.declare_dram_parameter("out", [b_loc, t, dim], F32, isOutput=True)

    with tile.TileContext(nc) as tc:
        with (
            tc.tile_pool(name="wq", bufs=n_dc) as p_wq,
            tc.tile_pool(name="wp", bufs=n_dc) as p_wp,
            tc.tile_pool(name="xbf", bufs=b_loc * n_dc) as p_x,
            tc.tile_pool(name="qk", bufs=b_loc * 2 * n_hp) as p_qk,
            tc.tile_pool(name="v", bufs=b_loc * n_tt) as p_v,
            tc.tile_pool(name="outT", bufs=b_loc * n_hp) as p_out,
            tc.tile_pool(name="expT", bufs=2) as p_exp,
            tc.tile_pool(name="bias", bufs=1) as p_b,
            tc.tile_pool(name="y", bufs=6) as p_y,
            tc.tile_pool(name="pc", bufs=3) as p_pc,
            tc.tile_pool(name="dpk", bufs=4) as p_dpk,
            tc.tile_pool(name="rc", bufs=4) as p_rc,
            tc.tile_pool(name="sel", bufs=1) as p_sel,
            tc.tile_pool(name="warm", bufs=1) as p_warm,
            tc.tile_pool(name="psmm", bufs=2, space="PSUM") as ps_mm,
            tc.tile_pool(name="pssc", bufs=2, space="PSUM") as ps_sc,
            tc.tile_pool(name="pso", bufs=2, space="PSUM") as ps_o,
        ):
            # ---- DMA preamble: emit in need-order so the first matmul can
            # start ~1.5us in and the qkv pipeline is DMA-paced, not blocked.
            b_qk_sb = p_b.tile([128, 2 * n_hp], F32, tag="bqk")
            nc.sync.dma_start(b_qk_sb[:], bqk_d[:, :])

            wq_bf = [p_wq.tile([128, o3], BF16, tag="wq", name=f"wq{dc}")
                     for dc in range(n_dc)]
            wp_bf = [p_wp.tile([128, dim], BF16, tag="wp", name=f"wp{dc}")
                     for dc in range(n_dc)]
            x_bf = [[p_x.tile([128, t], BF16, tag="xbf", name=f"x{b}_{dc}")
                     for dc in range(n_dc)] for b in range(b_loc)]

            wl = p_warm.tile([64, 64], BF16, tag="wlhs")
            nc.vector.memset(wl[:], 0.0)
            warm_in = p_warm.tile([1, 8], F32, tag="warm")
            nc.vector.memset(warm_in[:], 0.0)
            nc.scalar.activation(warm_in[:], warm_in[:],
                                 mybir.ActivationFunctionType.Exp)
            # PE warmup burst: ~5us of throwaway matmuls while the input DMAs
            # land. Drives the HAM activity monitor to K=8/8 BEFORE the real
            # stream starts (the clock-gate window is free-running, so without
            # this the first ~30us run at 1.2GHz with a phase-dependent ramp).
            wr = p_warm.tile([64, TC], BF16, tag="wrhs")
            nc.vector.memset(wr[:], 0.0)
            wp_ps = ps_mm.tile([64, TC], F32, tag="psmm", name="warm_ps")
            for _ in range(14):
                nc.tensor.matmul(wp_ps[:], lhsT=wl[:], rhs=wr[:],
                                 start=True, stop=True)
            # DMA need-order: x(b0) + the exact 128-col weight slices for the
            # first head pair (q0/k0) first so attention starts ~10us in,
            # then the v weights (woven into head-pair 0), then the rest.
            # The startup is DMA-LATENCY-serialized, so the critical prefix
            # alternates between both HWDGE queues (scalar is empty here).
            pre_qs = [nc.sync, nc.scalar]
            for dc in range(n_dc):
                pre_qs[dc % 2].dma_start(x_bf[0][dc][:],
                                         xT_d[0, dc * 128:(dc + 1) * 128, :])
                pre_qs[1 - dc % 2].dma_start(
                    wq_bf[dc][:, 0:128],
                    wq_d[dc * 128:(dc + 1) * 128, 0:128])
            for dc in range(n_dc):
                pre_qs[dc % 2].dma_start(
                    wq_bf[dc][:, dim:dim + 128],
                    wq_d[dc * 128:(dc + 1) * 128, dim:dim + 128])
            # v weights + bias (consumed by the v-weave inside head pair 0)
            b_v_sb = p_b.tile([128, dim], F32, tag="bv")
            nc.sync.dma_start(b_v_sb[:], bv_d[:, :])
            for dc in range(n_dc):
                nc.sync.dma_start(wq_bf[dc][:, 2 * dim:o3],
                                  wq_d[dc * 128:(dc + 1) * 128, 2 * dim:o3])
            # remaining q/k sections (consumed by head pairs 1..5)
            for dc in range(n_dc):
                nc.sync.dma_start(wq_bf[dc][:, 128:dim],
                                  wq_d[dc * 128:(dc + 1) * 128, 128:dim])
            for dc in range(n_dc):
                nc.sync.dma_start(wq_bf[dc][:, dim + 128:2 * dim],
                                  wq_d[dc * 128:(dc + 1) * 128, dim + 128:2 * dim])
            for b in range(1, b_loc):
                for dc in range(n_dc):
                    nc.sync.dma_start(x_bf[b][dc][:],
                                      xT_d[b, dc * 128:(dc + 1) * 128, :])
            b_p_sb = p_b.tile([128, dim], F32, tag="bp")
            nc.sync.dma_start(b_p_sb[:], bp_d[:, :])
            for dc in range(n_dc):
                nc.sync.dma_start(wp_bf[dc][:], wp_d[dc * 128:(dc + 1) * 128, :])

            # ones selector: lhsT of the K=1 "broadcast matmul" that
            # replicates a reciprocal row across 64 PSUM partitions.
            # bf16: fp32 matmuls run 2-pass (LOW_HIGH) at ~2.5x the cost.
            sel = p_sel.tile([1, 64], BF16, tag="sel")
            nc.vector.memset(sel[:], 1.0)

            # ---- persistent SBUF tiles ----
            qk = [[p_qk.tile([128, t], BF16, tag="qk", name=f"qk{b}_{ot}")
                   for ot in range(2 * n_hp)] for b in range(b_loc)]
            v_tiles = [[None] * n_tt for _ in range(b_loc)]
            outT = [[None] * n_hp for _ in range(b_loc)]

            # ---- stage A/C group emitters (each: 6 MMs + DVE epilogue) ----
            def qk_group(b, ot, i0):
                ps = ps_mm.tile([128, TC], F32, tag="psmm", name="ps_qk")
                for dc in range(n_dc):
                    nc.tensor.matmul(
                        ps[:],
                        lhsT=wq_bf[dc][:, ot * 128:(ot + 1) * 128],
                        rhs=x_bf[b][dc][:, i0:i0 + TC],
                        start=(dc == 0),
                        stop=(dc == n_dc - 1),
                    )
                nc.vector.tensor_scalar_add(
                    qk[b][ot][:, i0:i0 + TC], ps[:], b_qk_sb[:, ot:ot + 1]
                )

            def v_group(b, tt, half):
                if half == 0:
                    vt = p_v.tile([128, nh * 65], BF16, tag="v", name=f"v{b}_{tt}")
                    v_tiles[b][tt] = vt
                    v3 = vt[:].rearrange("p (h c) -> p h c", c=65)
                    nc.vector.memset(v3[:, :, 64:65], 1.0)
                vt = v_tiles[b][tt]
                v3 = vt[:].rearrange("p (h c) -> p h c", c=65)
                o0 = half * TC
                oc = min(TC, dim - o0)
                h0 = o0 // 64
                nhc = oc // 64
                ps = ps_mm.tile([128, oc], F32, tag="psmm", name="ps_v")
                for dc in range(n_dc):
                    nc.tensor.matmul(
                        ps[:],
                        lhsT=x_bf[b][dc][:, tt * 128:(tt + 1) * 128],
                        rhs=wq_bf[dc][:, 2 * dim + o0:2 * dim + o0 + oc],
                        start=(dc == 0),
                        stop=(dc == n_dc - 1),
                    )
                nc.vector.tensor_add(
                    v3[:, h0:h0 + nhc, 0:64],
                    ps[:].rearrange("p (h c) -> p h c", c=64),
                    b_v_sb[:, o0:o0 + oc].rearrange("p (h c) -> p h c", c=64),
                )

            # All stores on the sync queue. Never gpsimd: a store waiting its
            # DVE add would sit AHEAD of normalize broadcasts in the gpsimd
            # queue, while the DVE waits on those broadcasts -> 5us+ stalls.
            # Never scalar: stores must not delay the exp stream.
            store_qs = [nc.sync]
            store_i = [0]

            def c_group(b, tt, e0, store_q=None, pool=None):
                ec = min(TC, dim - e0)
                ps = (pool or ps_mm).tile([128, ec],
                                          F32,
                                          tag="psmm" if pool is None else "pssc",
                                          name="ps_c")
                for hp in range(n_hp):
                    nc.tensor.matmul(
                        ps[:],
                        lhsT=outT[b][hp][:, tt * 128:(tt + 1) * 128],
                        rhs=wp_bf[hp][:, e0:e0 + ec],
                        start=(hp == 0),
                        stop=(hp == n_hp - 1),
                    )
                yt = p_y.tile([128, ec], F32, tag="y", name="yt")
                nc.vector.tensor_add(yt[:], ps[:], b_p_sb[:, e0:e0 + ec])
                # round-robin output stores over two DMA queues: the tail
                # would otherwise serialize on one queue's ring.
                if store_q is None:
                    store_q = store_qs[store_i[0] % len(store_qs)]
                    store_i[0] += 1
                store_q.dma_start(
                    out_d[b, tt * 128:(tt + 1) * 128, e0:e0 + ec], yt[:]
                )

            # ---- filler pump: feeds the PE independent matmul groups while
            # ScalarE chews on exp, so HAM never sees a PE-idle window.
            pending = []
            pend_i = [0]

            def pump(k):
                n = min(k, len(pending) - pend_i[0])
                for _ in range(n):
                    pending[pend_i[0]]()
                    pend_i[0] += 1

            def pump_until(idx):
                while pend_i[0] < idx:
                    pending[pend_i[0]]()
                    pend_i[0] += 1

            def set_pending(groups):
                assert pend_i[0] == len(pending), \
                    f"filler leftover: {len(pending) - pend_i[0]}"
                pending.clear()
                pending.extend(groups)
                pend_i[0] = 0

            def flush_pending():
                pump(len(pending))

            # ---- stage B: attention for one head pair, with filler weaving.
            # `budget` filler groups are pumped during this head pair, spread
            # over pump slots after each exp (before the dependent PV) so the
            # PE always has independent work while ScalarE catches up.
            # `per_jt(jt)` (if set) emits dependent groups (the v tiles this
            # very head pair consumes) right before PV(jt). `pre` is a queue
            # index that must be drained before this head pair's matmuls.
            deferred = []

            def flush_deferred(keep=0):
                while len(deferred) > keep:
                    deferred.pop(0)()

            def attn(b, hp, budget=0, per_jt=None, pre=None):
                if pre is not None:
                    pump_until(pre)
                q_t = qk[b][hp]
                k_t = qk[b][n_hp + hp]
                if outT[b][hp] is None:
                    outT[b][hp] = p_out.tile([128, t], BF16, tag="outT",
                                             name=f"o{b}_{hp}")
                o_tile = outT[b][hp]
                # per-i0 pump counts over slots (jt=1,3,5,7, end-of-i0);
                # remainder goes to the jt7 and jt1 slots first -- they cover
                # the unit-boundary and pipeline-fill bubbles respectively.
                n_slots = 2 * 5
                rem_order = [3, 8, 0, 5, 1, 6, 2, 7, 4, 9]
                sched = [budget // n_slots] * n_slots
                for k in range(budget % n_slots):
                    sched[rem_order[k]] += 1
                for ic, i0 in enumerate((0, TC)):
                    # run normalize-finish ops two units late: their broadcast
                    # DMAs (multi-us SWDGE latency) completed a full unit of
                    # real time ago, so these never stall the DVE queue.
                    flush_deferred(keep=1)
                    po = [
                        ps_o.tile([65, TC], F32, tag="pso", name="po0"),
                        ps_o.tile([65, TC], F32, tag="pso", name="po1"),
                    ]
                    for jt in range(n_tt):
                        sc = ps_sc.tile([128, 2 * TC], F32, tag="pssc", name="sc")
                        # two heads of the pair: row-tiled concurrent matmuls
                        # (tile_position auto-derives from base_partition 0/64)
                        for sub in range(2):
                            nc.tensor.matmul(
                                sc[:, sub * TC:(sub + 1) * TC],
                                lhsT=k_t[sub * 64:(sub + 1) * 64,
                                         jt * 128:(jt + 1) * 128],
                                rhs=q_t[sub * 64:(sub + 1) * 64, i0:i0 + TC],
                                start=True,
                                stop=True,
                            )
                        et = p_exp.tile([128, 2 * TC], BF16, tag="expT", name="et")
                        nc.scalar.activation(
                            et[:], sc[:],
                            mybir.ActivationFunctionType.Exp,
                            scale=scale,
                        )
                        if per_jt is not None:
                            per_jt(jt)
                        elif jt % 2 == 1:
                            # jt==7 also takes the former end-of-unit share:
                            # all pumps sit between QK and PV so filler never
                            # delays the next unit's first QK.
                            pump(sched[ic * 5 + jt // 2]
                                 + (sched[ic * 5 + 4] if jt == 7 else 0))
                        for sub in range(2):
                            h = 2 * hp + sub
                            nc.tensor.matmul(
                                po[sub][:],
                                lhsT=v_tiles[b][jt][:, h * 65:(h + 1) * 65],
                                rhs=et[:, sub * TC:(sub + 1) * TC],
                                start=(jt == 0),
                                stop=(jt == n_tt - 1),
                            )
                    # Normalize, stage 1 (immediate): DVE copies free the po
                    # banks fast -- numerators of both heads packed into one
                    # [128, 512] tile (head 1 moved to partitions 64..127, so
                    # the final multiply writes outT in one op), den rows
                    # packed into [2, 512]. Then ONE den-broadcast DMA starts.
                    # all four po-freeing copies FIRST (each po bank frees the
                    # moment its two copies land; recips/casts don't gate the
                    # next unit's PVs), then the recip+cast pairs.
                    pc = p_pc.tile([128, TC], F32, tag="pc", name="pc")
                    dpks = []
                    for sub in range(2):
                        nc.vector.tensor_copy(
                            pc[sub * 64:(sub + 1) * 64, :], po[sub][0:64, :]
                        )
                        # den row to SBUF partition 0 (the custom recip uop
                        # needs a partition-0-aligned SBUF input)
                        dpk = p_dpk.tile([1, TC], F32, tag="dpk", name="dpk")
                        nc.vector.tensor_copy(dpk[:], po[sub][64:65, :])
                        dpks.append(dpk)
                    rcs = []
                    for sub in range(2):
                        rc = p_rc.tile([1, TC], F32, tag="rc", name="rc")
                        nc.vector.reciprocal_approx_fast(rc[:], dpks[sub][:])
                        # bf16 for the broadcast matmul (fp32 MMs are 2-pass)
                        rcb = p_rc.tile([1, TC], BF16, tag="rcbf", name="rcb")
                        nc.vector.tensor_copy(rcb[:], rc[:])
                        rcs.append(rcb)

                    # Normalize, stage 2 (deferred one unit): broadcast the
                    # two recip rows across partitions with K=1 col-tiled
                    # matmuls (~300ns on the PE; a DMA broadcast is
                    # single-SBUF-port-bound and takes 5us+), then multiply.
                    def fin(o_tile=o_tile, i0=i0, pc=pc, rcs=rcs):
                        rc_ps = ps_mm.tile([128, TC], F32, tag="psmm",
                                           name="rc_ps")
                        for sub in range(2):
                            nc.tensor.matmul(
                                rc_ps[sub * 64:(sub + 1) * 64, :],
                                lhsT=sel[:],
                                rhs=rcs[sub][:],
                                start=True,
                                stop=True,
                            )
                        nc.vector.tensor_mul(
                            o_tile[:, i0:i0 + TC], pc[:], rc_ps[:]
                        )

                    deferred.append(fin)

            # ================= master emission sequence =================
            # A(b0) minimal prefix: q0/k0 so head-pair 0 scores can start.
            for ot in (0, n_hp):
                for i0 in (0, TC):
                    qk_group(0, ot, i0)

            # v-weave: emits v tiles 2-per-jt so v[jt] always precedes the
            # PV(jt) of the same head pair that consumes it.
            def make_v_weaver(b):
                cnt = [0]

                def weave(jt):
                    while cnt[0] < min(2 * (jt + 1), 2 * n_tt):
                        v_group(b, cnt[0] // 2, cnt[0] % 2)
                        cnt[0] += 1
                return weave

            attn(0, 0, per_jt=make_v_weaver(0))

            # B(b0, hp1..5) filler queue: q/k(b0) for hp 1..5 (with drain
            # markers before their consumers) + ALL of A(b1) including v(b1),
            # so batch 1's attention can start with zero prerequisite work.
            q1 = []
            for hpp in range(1, n_hp):
                for i0 in (0, TC):
                    q1.append(lambda o=hpp, i0=i0: qk_group(0, o, i0))
                    q1.append(lambda o=n_hp + hpp, i0=i0: qk_group(0, o, i0))
            for tt in range(n_tt):
                for half in (0, 1):
                    q1.append(lambda tt=tt, half=half: v_group(1, tt, half))
            set_pending(q1)
            nb = len(q1)
            for hp in range(1, n_hp):
                attn(0, hp, budget=nb // 5 + (1 if hp - 1 < nb % 5 else 0),
                     pre=4 * hp)
            flush_pending()

            # B(b1): all of A(b1)'s q/k (with drain markers: qk(1,hp) must
            # land before attn(1,hp) reads it) + C(b0).
            q2 = []
            for hpp in range(n_hp):
                for i0 in (0, TC):
                    q2.append(lambda o=hpp, i0=i0: qk_group(1, o, i0))
                    q2.append(lambda o=n_hp + hpp, i0=i0: qk_group(1, o, i0))
            cg0 = [(tt, e0) for tt in range(n_tt) for e0 in range(0, dim, TC)]
            seam = cg0[-4:]  # reserved for the tail seam, outside the queue
            q2 += [(lambda tt=tt, e0=e0: c_group(0, tt, e0))
                   for tt, e0 in cg0[:-4]]
            set_pending(q2)
            # budgets weighted toward late head pairs (the queue otherwise
            # runs dry exactly where the tail seam needs PE cover)
            budgets = [4, 4, 5, 6, 8, 9]
            assert sum(budgets) == len(q2)
            for hp in range(n_hp):
                attn(1, hp, budget=budgets[hp], pre=4 * (hp + 1))
            flush_pending()

            # Tail seam: while the last unit's normalize chain drains on the
            # DVE, the PE needs dependency-free work. Two groups use the mm
            # psum ring; two use the (now idle) score-psum buffers, so none
            # of the four waits on a DVE add queued behind the chain.
            for gi, (tt, e0) in enumerate(seam):
                c_group(0, tt, e0, pool=(ps_sc if gi >= 2 else None))
            flush_deferred()
            for tt in range(n_tt):
                for e0 in range(0, dim, TC):
                    c_group(1, tt, e0, store_q=nc.sync)

    nc.compile()
    return nc


def make_in_maps(x, w_qkv, b_qkv, w_proj, b_proj):
    import ml_dtypes

    bf16 = np.dtype(ml_dtypes.bfloat16)
    x = np.asarray(x, dtype=np.float32)
    w_qkvT = np.ascontiguousarray(np.asarray(w_qkv, np.float32).T).astype(bf16)
    w_projT = np.ascontiguousarray(np.asarray(w_proj, np.float32).T).astype(bf16)
    b_qkv = np.asarray(b_qkv, np.float32)
    b_qkT = np.ascontiguousarray(b_qkv[:2 * DIM].reshape(2 * DIM // 128, 128).T)
    b_v = np.ascontiguousarray(np.broadcast_to(b_qkv[2 * DIM:], (128, DIM)))
    b_p = np.ascontiguousarray(np.broadcast_to(np.asarray(b_proj, np.float32), (128, DIM)))
    in_maps = []
    for c in range(N_CORES):
        xs = x[c * B_LOC:(c + 1) * B_LOC]
        xT = np.ascontiguousarray(xs.transpose(0, 2, 1)).astype(bf16)
        in_maps.append({
            "xT": xT,
            "w_qkvT": w_qkvT,
            "w_projT": w_projT,
            "b_qkT": b_qkT,
            "b_v": b_v,
            "b_proj": b_p,
        })
    return in_maps


_NC_CACHE = {}


def _get_nc():
    if "nc" not in _NC_CACHE:
        _NC_CACHE["nc"] = build_nc()
    return _NC_CACHE["nc"]


def run(x, w_qkv, b_qkv, w_proj, b_proj, **rb_kwargs):
    nc = _get_nc()
    in_maps = make_in_maps(x, w_qkv, b_qkv, w_proj, b_proj)
    res = run_bass_kernel_spmd(nc, in_maps, core_ids=list(range(N_CORES)), **rb_kwargs)
    out = np.concatenate([r["out"] for r in res.results], axis=0)
    return out.astype(np.float32), res


def kernel(x, w_qkv, b_qkv, w_proj, b_proj):
    out, _ = run(x, w_qkv, b_qkv, w_proj, b_proj)
    return out


# revision 72
# speedup vs baseline: 1.1961x; 1.1961x over previous
"""Multi-head attention (B=16, T=1024, D=768, H=12) on 8 TRN2 NeuronCores.

Strategy: pure data parallelism over the batch dim (2 batches per core, no
collectives). Per core, a Tile kernel computes the full attention block:

  qkv = x @ Wqkv.T + b            (q,k produced transposed [o, T]; v normal [T, o])
  scoresT = (k_h qT_h) * scale    ([j, i] layout; the 2 heads of a pair run as
                                   row-tiled CONCURRENT matmuls into the two
                                   halves of one 2-bank PSUM tile)
  expT = exp(scoresT)             (one [128,1024] ACTIVATE per j-tile covers both
                                   heads: (N+352)/1.2 amortizes the 352-cyc cost)
  outT_aug = v_aug.T? PV matmul   (v with appended ones col -> rows 0..63 = out,
                                   row 64 = softmax denominators)
  outT = outT_aug[:64] / sums     (reciprocal_approx_fast + bcast DMA + DVE mul)
  y = outT.T @ WprojT + b         (normal [t, e] layout, contiguous DMA out)

Perf architecture (vs the naive staged version): the attention phase is
ScalarE(exp)-bound, which starves the PE and lets the HAM clock gate
re-throttle it to 1.2 GHz. To keep the PE saturated at 2.4 GHz, independent
matmul "filler groups" (the OTHER batch element's qkv projection, and the
previous batch's output projection) are woven into the attention phase's
tensor-engine stream via a pump queue. The serial softmax-normalize chain
uses reciprocal_approx_fast (~5x faster than InstReciprocal) and runs on
DVE/GpSimd/DMA entirely off the PE critical path.
"""

import numpy as np

import concourse.bass as bass
import concourse.mybir as mybir
import concourse.tile as tile
from concourse import bacc
from concourse.bass_utils import run_bass_kernel_spmd

F32 = mybir.dt.float32
BF16 = mybir.dt.bfloat16

N_CORES = 8
B = 16
T = 1024
NH = 12
HD = 64
DIM = NH * HD
B_LOC = B // N_CORES
TC = 512  # free-dim chunk (one PSUM bank of f32)


def build_nc(b_loc=B_LOC, t=T, nh=NH):
    assert nh % 2 == 0
    dim = nh * HD
    o3 = 3 * dim
    n_dc = dim // 128      # contraction chunks over dim
    n_hp = nh // 2         # head pairs
    n_tt = t // 128        # t tiles
    scale = HD ** -0.5

    nc = bacc.Bacc()

    xT_d = nc.declare_dram_parameter("xT", [b_loc, dim, t], BF16, isOutput=False)
    wq_d = nc.declare_dram_parameter("w_qkvT", [dim, o3], BF16, isOutput=False)
    wp_d = nc.declare_dram_parameter("w_projT", [dim, dim], BF16, isOutput=False)
    bqk_d = nc.declare_dram_parameter("b_qkT", [128, 2 * n_hp], F32, isOutput=False)
    bv_d = nc.declare_dram_parameter("b_v", [128, dim], F32, isOutput=False)
    bp_d = nc.declare_dram_parameter("b_proj", [128, dim], F32, isOutput=False)
    out_d = nc.declare_dram_parameter("out", [b_loc, t, dim], F32, isOutput=True)

    with tile.TileContext(nc) as tc:
        with (
            tc.tile_pool(name="wq", bufs=n_dc) as p_wq,
            tc.tile_pool(name="wp", bufs=n_dc) as p_wp,
            tc.tile_pool(name="xbf", bufs=b_loc * n_dc) as p_x,
            tc.tile_pool(name="qk", bufs=b_loc * 2 * n_hp) as p_qk,
            tc.tile_pool(name="v", bufs=b_loc * n_tt) as p_v,
            tc.tile_pool(name="outT", bufs=b_loc * n_hp) as p_out,
            tc.tile_pool(name="expT", bufs=2) as p_exp,
            tc.tile_pool(name="bias", bufs=1) as p_b,
            tc.tile_pool(name="y", bufs=6) as p_y,
            tc.tile_pool(name="pc", bufs=3) as p_pc,
            tc.tile_pool(name="dpk", bufs=4) as p_dpk,
            tc.tile_pool(name="rc", bufs=4) as p_rc,
            tc.tile_pool(name="sel", bufs=1) as p_sel,
            tc.tile_pool(name="warm", bufs=1) as p_warm,
            tc.tile_pool(name="psmm", bufs=2, space="PSUM") as ps_mm,
            tc.tile_pool(name="pssc", bufs=2, space="PSUM") as ps_sc,
            tc.tile_pool(name="pso", bufs=2, space="PSUM") as ps_o,
        ):
            # ---- DMA preamble: emit in need-order so the first matmul can
            # start ~1.5us in and the qkv pipeline is DMA-paced, not blocked.
            b_qk_sb = p_b.tile([128, 2 * n_hp], F32, tag="bqk")
            nc.sync.dma_start(b_qk_sb[:], bqk_d[:, :])

            wq_bf = [p_wq.tile([128, o3], BF16, tag="wq", name=f"wq{dc}")
                     for dc in range(n_dc)]
            wp_bf = [p_wp.tile([128, dim], BF16, tag="wp", name=f"wp{dc}")
                     for dc in range(n_dc)]
            x_bf = [[p_x.tile([128, t], BF16, tag="xbf", name=f"x{b}_{dc}")
                     for dc in range(n_dc)] for b in range(b_loc)]

            wl = p_warm.tile([64, 64], BF16, tag="wlhs")
            nc.vector.memset(wl[:], 0.0)
            warm_in = p_warm.tile([1, 8], F32, tag="warm")
            nc.vector.memset(warm_in[:], 0.0)
            nc.scalar.activation(warm_in[:], warm_in[:],
                                 mybir.ActivationFunctionType.Exp)
            # PE warmup burst: ~5us of throwaway matmuls while the input DMAs
            # land. Drives the HAM activity monitor to K=8/8 BEFORE the real
            # stream starts (the clock-gate window is free-running, so without
            # this the first ~30us run at 1.2GHz with a phase-dependent ramp).
            wr = p_warm.tile([64, TC], BF16, tag="wrhs")
            nc.vector.memset(wr[:], 0.0)
            wp_ps = ps_mm.tile([64, TC], F32, tag="psmm", name="warm_ps")
            for _ in range(14):
                nc.tensor.matmul(wp_ps[:], lhsT=wl[:], rhs=wr[:],
                                 start=True, stop=True)
            # DMA need-order: x(b0) + the exact 128-col weight slices for the
            # first head pair (q0/k0) first so attention starts ~10us in,
            # then the v weights (woven into head-pair 0), then the rest.
            # The startup is DMA-LATENCY-serialized, so the critical prefix
            # alternates between both HWDGE queues (scalar is empty here).
            pre_qs = [nc.sync, nc.scalar]
            for dc in range(n_dc):
                pre_qs[dc % 2].dma_start(x_bf[0][dc][:],
                                         xT_d[0, dc * 128:(dc + 1) * 128, :])
                pre_qs[1 - dc % 2].dma_start(
                    wq_bf[dc][:, 0:128],
                    wq_d[dc * 128:(dc + 1) * 128, 0:128])
            for dc in range(n_dc):
                pre_qs[dc % 2].dma_start(
                    wq_bf[dc][:, dim:dim + 128],
                    wq_d[dc * 128:(dc + 1) * 128, dim:dim + 128])
            # v weights + bias (consumed by the v-weave inside head pair 0)
            b_v_sb = p_b.tile([128, dim], F32, tag="bv")
            nc.sync.dma_start(b_v_sb[:], bv_d[:, :])
            for dc in range(n_dc):
                nc.sync.dma_start(wq_bf[dc][:, 2 * dim:o3],
                                  wq_d[dc * 128:(dc + 1) * 128, 2 * dim:o3])
            # remaining q/k sections (consumed by head pairs 1..5)
            for dc in range(n_dc):
                nc.sync.dma_start(wq_bf[dc][:, 128:dim],
                                  wq_d[dc * 128:(dc + 1) * 128, 128:dim])
            for dc in range(n_dc):
                nc.sync.dma_start(wq_bf[dc][:, dim + 128:2 * dim],
                                  wq_d[dc * 128:(dc + 1) * 128, dim + 128:2 * dim])
            for b in range(1, b_loc):
                for dc in range(n_dc):
                    nc.sync.dma_start(x_bf[b][dc][:],
                                      xT_d[b, dc * 128:(dc + 1) * 128, :])
            b_p_sb = p_b.tile([128, dim], F32, tag="bp")
            nc.sync.dma_start(b_p_sb[:], bp_d[:, :])
            for dc in range(n_dc):
                nc.sync.dma_start(wp_bf[dc][:], wp_d[dc * 128:(dc + 1) * 128, :])

            # ones selector: lhsT of the K=1 "broadcast matmul" that
            # replicates a reciprocal row across 64 PSUM partitions.
            # bf16: fp32 matmuls run 2-pass (LOW_HIGH) at ~2.5x the cost.
            sel = p_sel.tile([1, 64], BF16, tag="sel")
            nc.vector.memset(sel[:], 1.0)

            # ---- persistent SBUF tiles ----
            qk = [[p_qk.tile([128, t], BF16, tag="qk", name=f"qk{b}_{ot}")
                   for ot in range(2 * n_hp)] for b in range(b_loc)]
            v_tiles = [[None] * n_tt for _ in range(b_loc)]
            outT = [[None] * n_hp for _ in range(b_loc)]

            # ---- stage A/C group emitters (each: 6 MMs + DVE epilogue) ----
            def qk_group(b, ot, i0):
                ps = ps_mm.tile([128, TC], F32, tag="psmm", name="ps_qk")
                for dc in range(n_dc):
                    nc.tensor.matmul(
                        ps[:],
                        lhsT=wq_bf[dc][:, ot * 128:(ot + 1) * 128],
                        rhs=x_bf[b][dc][:, i0:i0 + TC],
                        start=(dc == 0),
                        stop=(dc == n_dc - 1),
                    )
                nc.vector.tensor_scalar_add(
                    qk[b][ot][:, i0:i0 + TC], ps[:], b_qk_sb[:, ot:ot + 1]
                )

            def v_group(b, tt, half):
                if half == 0:
                    vt = p_v.tile([128, nh * 65], BF16, tag="v", name=f"v{b}_{tt}")
                    v_tiles[b][tt] = vt
                    v3 = vt[:].rearrange("p (h c) -> p h c", c=65)
                    nc.vector.memset(v3[:, :, 64:65], 1.0)
                vt = v_tiles[b][tt]
                v3 = vt[:].rearrange("p (h c) -> p h c", c=65)
                o0 = half * TC
                oc = min(TC, dim - o0)
                h0 = o0 // 64
                nhc = oc // 64
                ps = ps_mm.tile([128, oc], F32, tag="psmm", name="ps_v")
                for dc in range(n_dc):
                    nc.tensor.matmul(
                        ps[:],
                        lhsT=x_bf[b][dc][:, tt * 128:(tt + 1) * 128],
                        rhs=wq_bf[dc][:, 2 * dim + o0:2 * dim + o0 + oc],
                        start=(dc == 0),
                        stop=(dc == n_dc - 1),
                    )
                nc.vector.tensor_add(
                    v3[:, h0:h0 + nhc, 0:64],
                    ps[:].rearrange("p (h c) -> p h c", c=64),
                    b_v_sb[:, o0:o0 + oc].rearrange("p (h c) -> p h c", c=64),
                )

            # All stores on the sync queue. Never gpsimd: a store waiting its
            # DVE add would sit AHEAD of normalize broadcasts in the gpsimd
            # queue, while the DVE waits on those broadcasts -> 5us+ stalls.
            # Never scalar: stores must not delay the exp stream.
            store_qs = [nc.sync]
            store_i = [0]

            def c_group(b, tt, e0, store_q=None, pool=None):
                ec = min(TC, dim - e0)
                ps = (pool or ps_mm).tile([128, ec],
                                          F32,
                                          tag="psmm" if pool is None else "pssc",
                                          name="ps_c")
                for hp in range(n_hp):
                    nc.tensor.matmul(
                        ps[:],
                        lhsT=outT[b][hp][:, tt * 128:(tt + 1) * 128],
                        rhs=wp_bf[hp][:, e0:e0 + ec],
                        start=(hp == 0),
                        stop=(hp == n_hp - 1),
                    )
                yt = p_y.tile([128, ec], F32, tag="y", name="yt")
                nc.vector.tensor_add(yt[:], ps[:], b_p_sb[:, e0:e0 + ec])
                # round-robin output stores over two DMA queues: the tail
                # would otherwise serialize on one queue's ring.
                if store_q is None:
                    store_q = store_qs[store_i[0] % len(store_qs)]
                    store_i[0] += 1
                store_q.dma_start(
                    out_d[b, tt * 128:(tt + 1) * 128, e0:e0 + ec], yt[:]
                )

            # ---- filler pump: feeds the PE independent matmul groups while
            # ScalarE chews on exp, so HAM never sees a PE-idle window.
            pending = []
            pend_i = [0]

            def pump(k):
                n = min(k, len(pending) - pend_i[0])
                for _ in range(n):
                    pending[pend_i[0]]()
                    pend_i[0] += 1

            def pump_until(idx):
                while pend_i[0] < idx:
                    pending[pend_i[0]]()
                    pend_i[0] += 1

            def set_pending(groups):
                assert pend_i[0] == len(pending), \
                    f"filler leftover: {len(pending) - pend_i[0]}"
                pending.clear()
                pending.extend(groups)
                pend_i[0] = 0

            def flush_pending():
                pump(len(pending))

            # ---- stage B: attention for one head pair, with filler weaving.
            # `budget` filler groups are pumped during this head pair, spread
            # over pump slots after each exp (before the dependent PV) so the
            # PE always has independent work while ScalarE catches up.
            # `per_jt(jt)` (if set) emits dependent groups (the v tiles this
            # very head pair consumes) right before PV(jt). `pre` is a queue
            # index that must be drained before this head pair's matmuls.
            deferred = []

            def flush_deferred(keep=0):
                while len(deferred) > keep:
                    deferred.pop(0)()

            def attn(b, hp, budget=0, per_jt=None, pre=None):
                if pre is not None:
                    pump_until(pre)
                q_t = qk[b][hp]
                k_t = qk[b][n_hp + hp]
                if outT[b][hp] is None:
                    outT[b][hp] = p_out.tile([128, t], BF16, tag="outT",
                                             name=f"o{b}_{hp}")
                o_tile = outT[b][hp]
                # per-i0 pump counts over slots (jt=1,3,5,7, end-of-i0);
                # remainder goes to the jt7 and jt1 slots first -- they cover
                # the unit-boundary and pipeline-fill bubbles respectively.
                n_slots = 2 * 5
                rem_order = [3, 8, 0, 5, 1, 6, 2, 7, 4, 9]
                sched = [budget // n_slots] * n_slots
                for k in range(budget % n_slots):
                    sched[rem_order[k]] += 1
                for ic, i0 in enumerate((0, TC)):
                    # run normalize-finish ops two units late: their broadcast
                    # DMAs (multi-us SWDGE latency) completed a full unit of
                    # real time ago, so these never stall the DVE queue.
                    flush_deferred(keep=1)
                    po = [
                        ps_o.tile([65, TC], F32, tag="pso", name="po0"),
                        ps_o.tile([65, TC], F32, tag="pso", name="po1"),
                    ]
                    for jt in range(n_tt):
                        sc = ps_sc.tile([128, 2 * TC], F32, tag="pssc", name="sc")
                        # two heads of the pair: row-tiled concurrent matmuls
                        # (tile_position auto-derives from base_partition 0/64)
                        for sub in range(2):
                            nc.tensor.matmul(
                                sc[:, sub * TC:(sub + 1) * TC],
                                lhsT=k_t[sub * 64:(sub + 1) * 64,
                                         jt * 128:(jt + 1) * 128],
                                rhs=q_t[sub * 64:(sub + 1) * 64, i0:i0 + TC],
                                start=True,
                                stop=True,
                            )
                        et = p_exp.tile([128, 2 * TC], BF16, tag="expT", name="et")
                        nc.scalar.activation(
                            et[:], sc[:],
                            mybir.ActivationFunctionType.Exp,
                            scale=scale,
                        )
                        if per_jt is not None:
                            per_jt(jt)
                        elif jt % 2 == 1:
                            # jt==7 also takes the former end-of-unit share:
                            # all pumps sit between QK and PV so filler never
                            # delays the next unit's first QK.
                            pump(sched[ic * 5 + jt // 2]
                                 + (sched[ic * 5 + 4] if jt == 7 else 0))
                        for sub in range(2):
                            h = 2 * hp + sub
                            nc.tensor.matmul(
                                po[sub][:],
                                lhsT=v_tiles[b][jt][:, h * 65:(h + 1) * 65],
                                rhs=et[:, sub * TC:(sub + 1) * TC],
                                start=(jt == 0),
                                stop=(jt == n_tt - 1),
                            )
                    # Normalize, stage 1 (immediate): DVE copies free the po
                    # banks fast -- numerators of both heads packed into one
                    # [128, 512] tile (head 1 moved to partitions 64..127, so
                    # the final multiply writes outT in one op), den rows
                    # packed into [2, 512]. Then ONE den-broadcast DMA starts.
                    # all four po-freeing copies FIRST (each po bank frees the
                    # moment its two copies land; recips/casts don't gate the
                    # next unit's PVs), then the recip+cast pairs.
                    pc = p_pc.tile([128, TC], F32, tag="pc", name="pc")
                    dpks = []
                    for sub in range(2):
                        nc.vector.tensor_copy(
                            pc[sub * 64:(sub + 1) * 64, :], po[sub][0:64, :]
                        )
                        # den row to SBUF partition 0 (the custom recip uop
                        # needs a partition-0-aligned SBUF input)
                        dpk = p_dpk.tile([1, TC], F32, tag="dpk", name="dpk")
                        nc.vector.tensor_copy(dpk[:], po[sub][64:65, :])
                        dpks.append(dpk)
                    rcs = []
                    for sub in range(2):
                        rc = p_rc.tile([1, TC], F32, tag="rc", name="rc")
                        nc.vector.reciprocal_approx_fast(rc[:], dpks[sub][:])
                        # bf16 for the broadcast matmul (fp32 MMs are 2-pass)
                        rcb = p_rc.tile([1, TC], BF16, tag="rcbf", name="rcb")
                        nc.vector.tensor_copy(rcb[:], rc[:])
                        rcs.append(rcb)

                    # Normalize, stage 2 (deferred one unit): broadcast the
                    # two recip rows across partitions with K=1 col-tiled
                    # matmuls (~300ns on the PE; a DMA broadcast is
                    # single-SBUF-port-bound and takes 5us+), then multiply.
                    def fin(o_tile=o_tile, i0=i0, pc=pc, rcs=rcs):
                        rc_ps = ps_mm.tile([128, TC], F32, tag="psmm",
                                           name="rc_ps")
                        for sub in range(2):
                            nc.tensor.matmul(
                                rc_ps[sub * 64:(sub + 1) * 64, :],
                                lhsT=sel[:],
                                rhs=rcs[sub][:],
                                start=True,
                                stop=True,
                            )
                        nc.vector.tensor_mul(
                            o_tile[:, i0:i0 + TC], pc[:], rc_ps[:]
                        )

                    deferred.append(fin)

            # ================= master emission sequence =================
            # A(b0) minimal prefix: q0/k0 so head-pair 0 scores can start.
            for ot in (0, n_hp):
                for i0 in (0, TC):
                    qk_group(0, ot, i0)

            # v-weave: emits v tiles 2-per-jt so v[jt] always precedes the
            # PV(jt) of the same head pair that consumes it.
            def make_v_weaver(b):
                cnt = [0]

                def weave(jt):
                    while cnt[0] < min(2 * (jt + 1), 2 * n_tt):
                        v_group(b, cnt[0] // 2, cnt[0] % 2)
                        cnt[0] += 1
                return weave

            attn(0, 0, per_jt=make_v_weaver(0))

            # B(b0, hp1..5) filler queue: q/k(b0) for hp 1..5 (with drain
            # markers before their consumers) + ALL of A(b1) including v(b1),
            # so batch 1's attention can start with zero prerequisite work.
            q1 = []
            for hpp in range(1, n_hp):
                for i0 in (0, TC):
                    q1.append(lambda o=hpp, i0=i0: qk_group(0, o, i0))
                    q1.append(lambda o=n_hp + hpp, i0=i0: qk_group(0, o, i0))
            for tt in range(n_tt):
                for half in (0, 1):
                    q1.append(lambda tt=tt, half=half: v_group(1, tt, half))
            set_pending(q1)
            nb = len(q1)
            for hp in range(1, n_hp):
                attn(0, hp, budget=nb // 5 + (1 if hp - 1 < nb % 5 else 0),
                     pre=4 * hp)
            flush_pending()

            # B(b1): all of A(b1)'s q/k (with drain markers: qk(1,hp) must
            # land before attn(1,hp) reads it) + C(b0).
            q2 = []
            for hpp in range(n_hp):
                for i0 in (0, TC):
                    q2.append(lambda o=hpp, i0=i0: qk_group(1, o, i0))
                    q2.append(lambda o=n_hp + hpp, i0=i0: qk_group(1, o, i0))
            cg0 = [(tt, e0) for tt in range(n_tt) for e0 in range(0, dim, TC)]
            seam = cg0[-4:]  # reserved for the tail seam, outside the queue
            q2 += [(lambda tt=tt, e0=e0: c_group(0, tt, e0))
                   for tt, e0 in cg0[:-4]]
            set_pending(q2)
            # budgets weighted toward late head pairs (the queue otherwise
            # runs dry exactly where the tail seam needs PE cover)
            budgets = [4, 4, 5, 6, 8, 9]
            assert sum(budgets) == len(q2)
            for hp in range(n_hp):
                attn(1, hp, budget=budgets[hp], pre=4 * (hp + 1))
            flush_pending()

            # Tail seam: while the last unit's normalize chain drains on the
            # DVE, the PE needs dependency-free work. Two groups use the mm
            # psum ring; two use the (now idle) score-psum buffers, so none
            # of the four waits on a DVE add queued behind the chain.
            for gi, (tt, e0) in enumerate(seam):
                c_group(0, tt, e0, pool=(ps_sc if gi >= 2 else None))
            flush_deferred()
            cg1 = [(tt, e0) for tt in range(n_tt) for e0 in range(0, dim, TC)]
            for gi, (tt, e0) in enumerate(cg1):
                # the very last stores split across sync+scalar so the final
                # DMA completions overlap (scalar is exp-free by the tail)
                q = nc.scalar if gi == len(cg1) - 1 else nc.sync
                c_group(1, tt, e0, store_q=q)

    nc.compile()
    return nc


def make_in_maps(x, w_qkv, b_qkv, w_proj, b_proj):
    import ml_dtypes

    bf16 = np.dtype(ml_dtypes.bfloat16)
    x = np.asarray(x, dtype=np.float32)
    w_qkvT = np.ascontiguousarray(np.asarray(w_qkv, np.float32).T).astype(bf16)
    w_projT = np.ascontiguousarray(np.asarray(w_proj, np.float32).T).astype(bf16)
    b_qkv = np.asarray(b_qkv, np.float32)
    b_qkT = np.ascontiguousarray(b_qkv[:2 * DIM].reshape(2 * DIM // 128, 128).T)
    b_v = np.ascontiguousarray(np.broadcast_to(b_qkv[2 * DIM:], (128, DIM)))
    b_p = np.ascontiguousarray(np.broadcast_to(np.asarray(b_proj, np.float32), (128, DIM)))
    in_maps = []
    for c in range(N_CORES):
        xs = x[c * B_LOC:(c + 1) * B_LOC]
        xT = np.ascontiguousarray(xs.transpose(0, 2, 1)).astype(bf16)
        in_maps.append({
            "xT": xT,
            "w_qkvT": w_qkvT,
            "w_projT": w_projT,
            "b_qkT": b_qkT,
            "b_v": b_v,
            "b_proj": b_p,
        })
    return in_maps


_NC_CACHE = {}


def _get_nc():
    if "nc" not in _NC_CACHE:
        _NC_CACHE["nc"] = build_nc()
    return _NC_CACHE["nc"]


def run(x, w_qkv, b_qkv, w_proj, b_proj, **rb_kwargs):
    nc = _get_nc()
    in_maps = make_in_maps(x, w_qkv, b_qkv, w_proj, b_proj)
    res = run_bass_kernel_spmd(nc, in_maps, core_ids=list(range(N_CORES)), **rb_kwargs)
    out = np.concatenate([r["out"] for r in res.results], axis=0)
    return out.astype(np.float32), res


def kernel(x, w_qkv, b_qkv, w_proj, b_proj):
    out, _ = run(x, w_qkv, b_qkv, w_proj, b_proj)
    return out
